# revision 1
# baseline (speedup 1.0000x reference)
"""Trainium2 Bass kernel for ContextWindowPredictor.

Computation (per batch b):
    e1 = hidden[b][pairs[b,:,0]]          # (P, H) gather
    e2 = hidden[b][pairs[b,:,1]]          # (P, H) gather
    h  = gelu([e1 e2] @ W1 + b1)          # (P, H)
    out = h @ W2 + b2                     # (P, 2)

Sharding: data-parallel over batch, one batch element per NeuronCore.

Device strategy (v7), token-factored U/V with transposed stage 2 and
fp8 hi/lo DoubleRow stage-1 matmuls:
    h[p] = gelu(U[s0_p] + V[s1_p]),  U = hid @ W1[:H] + b1, V = hid @ W1[H:]

  stage 1: hid.T is prepared on the HOST (input marshalling), split into
           e4m3 hi + residual lo; W1 is scaled by 16 and split the same
           way.  Each psum group runs 12 fp8 DoubleRow passes (157 TF/s):
           8 type-1 passes (hid_hi, hid_lo) x (Wh, Wh) and 4 type-2
           passes (hid_hi[2i], hid_hi[2i+1]) x (Wl[2i], Wl[2i+1]) — the
           hi*lo cross term is dropped (~1e-4 relative).  This keeps
           bf16-level accuracy at 0.75x the bf16 cycle count.  U,V live
           in SBUF by h-half ([128, 16, 512]: token s -> partition s%128,
           rank-stripe s//128) holding 16x-scaled values; 16*b1 is folded
           into the U psum->SBUF copies (DVE); V copies on ACT.
  stage 2: SBUF-source TRANSPOSE-mode dma_gather pulls pair rows out of
           U/V into [h-partition, h-tile, pair] layout, 512 pairs per
           call (the SWDGE ring cannot take 1024-descriptor gathers), as
           soon as each half's U (resp. V) is written.  The h0 gathers/
           adds/gelus and the h1 e1-gathers all run UNDER the remaining
           stage-1 matmuls; SBUF pools are ordered so the tail tiles
           reuse addresses that die early.  e1+e2 on DVE in-place, Gelu
           (exact erf) on ACT in-place with scale=1/16 undoing the W1
           scaling, then W2 on the PE as [2, 512] matmuls (contraction
           over h on partitions) into a single 8-bank psum tile, copied
           and DMA'd out once.  b2 is added on the host while unsharding.
"""

import sys

import numpy as np

if "/opt/trn_rl_repo" not in sys.path:
    sys.path.insert(0, "/opt/trn_rl_repo")

B, S, H, P = 8, 2048, 1024, 4096
N_CORES = 8
ST = S // 128          # 16 token tiles
KT = H // 128          # 8 contraction tiles per W1 half
NI = 512               # pairs per W2 psum chunk
NC = P // NI           # 8 chunks
NQ = 8                 # gather waves (512 idx max per SWDGE gather)
QP = P // NQ           # 512 pairs per gather call

_CACHE: dict = {}


def _build(rw=(16,) * NQ, act_name: str = "Gelu"):
    import concourse.bacc as bacc
    import concourse.mybir as mybir
    from concourse.tile import TileContext

    dt = mybir.dt
    AF = mybir.ActivationFunctionType
    PM = mybir.MatmulPerfMode

    nc = bacc.Bacc("TRN2", target_bir_lowering=False)

    # hid8[p, st, kt, d, q]: d=0 -> e4m3(hid), d=1 -> e4m3 residual;
    # hid8[p, st, kt, d, q] ~ hid[st*128+q, kt*128+p] split hi/lo
    hid8 = nc.dram_tensor("hid8", [128, ST, KT, 2, 128], dt.float8e4,
                          kind="ExternalInput")
    # w1t1[p,half,hc,kt,d,j]: both d slots = e4m3(16*W1) block (kt, hc)
    w1t1 = nc.dram_tensor("w1t1", [128, 2, 2, KT, 2, 512], dt.float8e4,
                          kind="ExternalInput")
    # w1t2[p,half,hc,i,d,j]: slot d = e4m3 residual of 16*W1, k-tile 2i+d
    w1t2 = nc.dram_tensor("w1t2", [128, 2, 2, KT // 2, 2, 512], dt.float8e4,
                          kind="ExternalInput")
    b1r = nc.dram_tensor("b1r", [128, H], dt.bfloat16, kind="ExternalInput")
    # w2s[p, kt, o] = W2[kt*128+p, o]
    w2s = nc.dram_tensor("w2s", [128, KT, 2], dt.bfloat16, kind="ExternalInput")
    idx0 = nc.dram_tensor("idx0", [128, P // 16], dt.int16, kind="ExternalInput")
    idx1 = nc.dram_tensor("idx1", [128, P // 16], dt.int16, kind="ExternalInput")
    outT = nc.dram_tensor("outT", [2, P], dt.float32, kind="ExternalOutput")

    act_fn = getattr(AF, act_name)

    with TileContext(nc) as tc:
        with (
            tc.tile_pool(name="uv", bufs=1) as uvp,
            tc.tile_pool(name="cst", bufs=1) as cst,
            tc.tile_pool(name="ge0", bufs=1) as ge0p,
            tc.tile_pool(name="w1l", bufs=1) as w1lp,
        ):
            # ---- constants (loaded later where latency allows) ----
            i0s = cst.tile([128, P // 16], dt.int16, tag="i0s")
            i1s = cst.tile([128, P // 16], dt.int16, tag="i1s")
            b1s = cst.tile([128, H], dt.bfloat16, tag="b1s")
            w2t = cst.tile([128, KT, 2], dt.bfloat16, tag="w2t")

            usb = [uvp.tile([128, ST, 512], dt.bfloat16, tag=f"usb{h}",
                            name=f"usb{h}") for h in range(2)]
            vsb = [uvp.tile([128, ST, 512], dt.bfloat16, tag=f"vsb{h}",
                            name=f"vsb{h}") for h in range(2)]

            # h0 pair tiles; after in-place add + gelu these hold
            # gelu(hpre) h-half-0 until W2 reads them.
            e1h0 = [ge0p.tile([128, 4, QP], dt.bfloat16, tag=f"e1h0q{q}",
                              name=f"e1h0q{q}") for q in range(NQ)]
            # last W1 piece lives to the end of stage 1 — keep it out of the
            # recycled region so tail tiles don't inherit its WAR.
            w1p = {(1, 1): (
                w1lp.tile([128, KT, 2, 512], dt.float8e4, tag="w1pL1",
                          name="w1t1_1_1"),
                w1lp.tile([128, KT // 2, 2, 512], dt.float8e4, tag="w1pL2",
                          name="w1t2_1_1"),
            )}

            def gather(src, isrc, et, q):
                from concourse.bass import AP
                nc.gpsimd.dma_gather(
                    out_ap=et[:],
                    in_ap=src if isinstance(src, AP) else src[:],
                    idxs_ap=isrc[:, q * (QP // 16):(q + 1) * (QP // 16)],
                    num_idxs=QP,
                    num_idxs_reg=QP,
                    elem_size=512,
                    transpose=True,
                    sbuf_tokens_per_rank=128,
                    sbuf_free_dim_per_rank=1024,
                )

            # ================= stage 1 (+ h0 stage-2 front) =================
            # Pool open order controls SBUF placement: e20 (dead ~3/4 in) and
            # s1w (W1 pieces 0-2, dead by the U-h1 section) sit at the bottom
            # of the recycled region; hsb (read until the last matmul) at the
            # top, so the tail's e11/e21/lg pools land on early-dying space.
            with (
                tc.tile_pool(name="e20", bufs=2) as e20p,
                tc.tile_pool(name="s1w", bufs=1) as s1w,
                tc.tile_pool(name="s1h", bufs=1) as s1h,
                tc.tile_pool(name="ps1", bufs=4, space="PSUM") as ps1,
            ):
                # first W1 piece in chunks so matmul 0 starts early;
                # b1 right behind it (needed by the first U copy)
                t0a = s1w.tile([128, KT, 2, 512], dt.float8e4, tag="w1p00a",
                               name="w1t1_0_0")
                nc.scalar.dma_start(out=t0a[:, 0:4, :, :],
                                    in_=w1t1[:, 0, 0, 0:4, :, :])
                hsb = s1h.tile([128, ST, KT, 2, 128], dt.float8e4, tag="hsb")
                nc.sync.dma_start(out=hsb[:, 0, :, :, :], in_=hid8[:, 0, :, :, :])
                nc.scalar.dma_start(out=t0a[:, 4:8, :, :],
                                    in_=w1t1[:, 0, 0, 4:8, :, :])
                t0b = s1w.tile([128, KT // 2, 2, 512], dt.float8e4, tag="w1p00b",
                               name="w1t2_0_0")
                nc.scalar.dma_start(out=t0b[:], in_=w1t2[:, 0, 0, :, :, :])
                nc.scalar.dma_start(out=b1s[:], in_=b1r[:])
                w1p[(0, 0)] = (t0a, t0b)
                for st in range(1, ST):
                    nc.sync.dma_start(out=hsb[:, st, :, :, :],
                                      in_=hid8[:, st, :, :, :])
                for half, hc, tg in ((1, 0, "w1p10"), (0, 1, "w1p01")):
                    ta = s1w.tile([128, KT, 2, 512], dt.float8e4, tag=tg + "a",
                                  name=f"w1t1_{half}_{hc}")
                    nc.scalar.dma_start(out=ta[:], in_=w1t1[:, half, hc, :, :, :])
                    tb = s1w.tile([128, KT // 2, 2, 512], dt.float8e4,
                                  tag=tg + "b", name=f"w1t2_{half}_{hc}")
                    nc.scalar.dma_start(out=tb[:], in_=w1t2[:, half, hc, :, :, :])
                    w1p[(half, hc)] = (ta, tb)
                nc.scalar.dma_start(out=w1p[(1, 1)][0][:],
                                    in_=w1t1[:, 1, 1, :, :, :])
                nc.scalar.dma_start(out=w1p[(1, 1)][1][:],
                                    in_=w1t2[:, 1, 1, :, :, :])
                nc.sync.dma_start(out=i0s[:], in_=idx0[:])
                nc.sync.dma_start(out=i1s[:], in_=idx1[:])
                nc.sync.dma_start(out=w2t[:], in_=w2s[:])

                def s1_section(hc, half):
                    dsts = usb if half == 0 else vsb
                    ta, tb = w1p[(half, hc)]
                    for st in range(ST):
                        ps = ps1.tile([128, 512], dt.float32, tag="ps",
                                      name=f"ps_{hc}_{half}_{st}")
                        # type-1: (hid_hi, hid_lo) x (Wh, Wh), one per k-tile
                        for kt in range(KT):
                            nc.tensor.matmul(
                                ps[:],
                                hsb[:, st, kt, :, :],
                                ta[:, kt, :, :],
                                start=(kt == 0),
                                stop=False,
                                perf_mode=PM.DoubleRow,
                            )
                        # type-2: (hid_hi[2i], hid_hi[2i+1]) x (Wl[2i], Wl[2i+1])
                        for i in range(KT // 2):
                            nc.tensor.matmul(
                                ps[:],
                                hsb[:, st, 2 * i:2 * i + 2, 0, :],
                                tb[:, i, :, :],
                                start=False,
                                stop=(i == KT // 2 - 1),
                                perf_mode=PM.DoubleRow,
                            )
                        dst = dsts[hc][:, st, :]
                        if half == 0:
                            nc.vector.tensor_add(
                                dst, ps[:], b1s[:, hc * 512:(hc + 1) * 512]
                            )
                        else:
                            nc.scalar.activation(dst, ps[:], AF.Copy)

                s1_section(0, 0)
                # e1-h0 gathers fire once usb0 is written (~1/4 into stage 1)
                for q in range(NQ):
                    gather(usb[0], i0s, e1h0[q], q)
                s1_section(0, 1)
                # h0 e2-gathers + adds + gelus run under the h1 matmul sections
                for q in range(NQ):
                    e2t = e20p.tile([128, 4, QP], dt.bfloat16, tag="e2h0",
                                    name=f"e2h0q{q}")
                    gather(vsb[0], i1s, e2t, q)
                    nc.vector.tensor_add(e1h0[q][:], e1h0[q][:], e2t[:])
                    nc.scalar.activation(e1h0[q][:], e1h0[q][:], act_fn, scale=1.0 / 16.0)
                s1_section(1, 0)
                s1_section(1, 1)

            # ================= stage 2 tail: h1 + W2 =================
            with (
                tc.tile_pool(name="e11", bufs=1) as e11p,
                tc.tile_pool(name="e21", bufs=6) as e21p,
                tc.tile_pool(name="ps2", bufs=1, space="PSUM") as ps2p,
                tc.tile_pool(name="lg", bufs=1) as lgp,
            ):
                e1h1 = [e11p.tile([128, 4, QP], dt.bfloat16, tag=f"e1h1q{q}",
                                  name=f"e1h1q{q}") for q in range(NQ)]
                # e1-h1 gathers fire at U-h1 completion, under the V-h1 mms
                for q in range(NQ):
                    gather(usb[1], i0s, e1h1[q], q)
                # issue all e2-h1 gathers first so the Pool queue streams
                # them back-to-back once vsb1 lands
                e2h1 = []
                for q in range(NQ):
                    e2t = e21p.tile([128, 4, QP], dt.bfloat16, tag="e2h1",
                                    name=f"e2h1q{q}")
                    gather(vsb[1][:, 0:rw[q], :], i1s, e2t, q)
                    e2h1.append(e2t)
                # adds first, then gelus, then W2+copy: keeps the lgt psum
                # copies (which wait on W2) from head-of-line-blocking the
                # next wave's add on the in-order DVE queue.
                for q in range(NQ):
                    nc.vector.tensor_add(e1h1[q][:], e1h1[q][:], e2h1[q][:])
                for q in range(NQ):
                    nc.scalar.activation(e1h1[q][:], e1h1[q][:], act_fn, scale=1.0 / 16.0)
                # one 8-bank psum tile holds every chunk's logits (on
                # partitions 0-1); a single copy per half keeps the
                # per-chunk critical chain free of psum-copy blocking.
                # Dummy matmuls into partitions 64-127 keep the PE busy
                # across the boundary and between W2 chunks so it stays at
                # the 2.4 GHz p-state (it drops to 1.2 GHz on any idle);
                # they depend on late-stage tiles so the scheduler cannot
                # hoist them into the saturated stage-1 stream.
                ps2 = ps2p.tile([128, NC, NI], dt.float32, tag="ps2")

                lgt = lgp.tile([2, NC, NI], dt.float32, tag="lgt")
                for c in range(NC):
                    q = c * NQ // NC
                    cc = c - q * NC // NQ
                    sl = slice(cc * NI, (cc + 1) * NI)
                    for kt in range(KT):
                        hat = e1h0[q] if kt < 4 else e1h1[q]
                        nc.tensor.matmul(
                            ps2[0:2, c, :],
                            w2t[:, kt, :],
                            hat[:, kt % 4, sl],
                            start=(kt == 0),
                            stop=(kt == KT - 1),
                        )
                nc.vector.tensor_copy(lgt[:], ps2[0:2, :, :])
                nc.sync.dma_start(out=outT[:],
                                  in_=lgt[:].rearrange("o c n -> o (c n)"))

    nc.compile()
    return nc


def _get_nc(rw=(16,) * NQ):
    key = tuple(rw)
    if key not in _CACHE:
        _CACHE[key] = _build(key)
    return _CACHE[key]


def _wrap_idx(idx: np.ndarray) -> np.ndarray:
    """[P] index list -> [128, P//16] int16 layout dma_gather expects."""
    w = idx.astype(np.int16).reshape(P // 16, 16).T  # [16, P//16]
    return np.ascontiguousarray(np.tile(w, (8, 1)))  # [128, P//16]


def _make_in_maps(hidden_states, pairs, W1, b1, W2):
    import ml_dtypes

    bf16 = ml_dtypes.bfloat16
    e4 = ml_dtypes.float8_e4m3  # IEEE e4m3 (max 240) == TRN FP8_EXP4
    hs = np.asarray(hidden_states, dtype=np.float32)
    pairs_i = np.asarray(pairs).astype(np.int32)
    W1f = np.asarray(W1, dtype=np.float32) * 16.0
    wh = W1f.astype(e4)
    wl = (W1f - wh.astype(np.float32)).astype(e4)
    # [p, half, hc, kt, j] view of a [2H, H] matrix
    def pview(w):
        return w.reshape(2, KT, 128, 2, 512).transpose(2, 0, 3, 1, 4)
    whr = pview(wh)
    w1t1 = np.ascontiguousarray(
        np.stack([whr, whr], axis=4)  # both DoubleRow slots = Wh
    )
    wlr = pview(wl)  # [p, half, hc, kt, j], kt = 2i + d
    w1t2 = np.ascontiguousarray(
        wlr.reshape(128, 2, 2, KT // 2, 2, 512)
    )
    b1f = np.ascontiguousarray(
        np.broadcast_to((np.asarray(b1, dtype=np.float32) * 16.0)
                        .reshape(1, H), (128, H)).astype(bf16)
    )
    w2sv = np.ascontiguousarray(
        np.asarray(W2, dtype=np.float32).reshape(KT, 128, 2).transpose(1, 0, 2)
        .astype(bf16)
    )
    in_maps = []
    for c in range(N_CORES):
        hc32 = hs[c]
        hh = hc32.astype(e4)
        hl = (hc32 - hh.astype(np.float32)).astype(e4)
        hd = np.stack([hh, hl])  # [2, S, H]
        h8 = np.ascontiguousarray(
            hd.reshape(2, ST, 128, KT, 128).transpose(4, 1, 3, 0, 2)
        )
        in_maps.append(
            {
                "hid8": h8,
                "w1t1": w1t1,
                "w1t2": w1t2,
                "b1r": b1f,
                "w2s": w2sv,
                "idx0": _wrap_idx(pairs_i[c, :, 0]),
                "idx1": _wrap_idx(pairs_i[c, :, 1]),
            }
        )
    return in_maps


def kernel(hidden_states, pairs, W1, b1, W2, b2):
    from concourse.bass_utils import run_bass_kernel_spmd

    pairs_i = np.asarray(pairs).astype(np.int32)
    # sort each core's pairs by the e2 token so tail gather wave q only
    # touches the first rw[q] token stripes of V-h1 (progressive deps)
    perms = [np.argsort(pairs_i[c, :, 1], kind="stable")
             for c in range(N_CORES)]
    ps = np.stack([pairs_i[c][perms[c]] for c in range(N_CORES)])
    rw = tuple(
        int(min(16, max(1, int(ps[:, (q + 1) * QP - 1, 1].max()) // 128 + 1)))
        for q in range(NQ)
    )
    nc = _get_nc(rw)
    in_maps = _make_in_maps(hidden_states, ps, W1, b1, W2)
    res = run_bass_kernel_spmd(nc, in_maps, core_ids=list(range(N_CORES)))
    b2f = np.asarray(b2, dtype=np.float32).reshape(1, 2)
    out = np.empty((N_CORES, P, 2), np.float32)
    for c in range(N_CORES):
        out[c, perms[c]] = np.asarray(res.results[c]["outT"]).T + b2f
    return np.ascontiguousarray(out)


if __name__ == "__main__":
    rng = np.random.default_rng(0)
    hs = rng.standard_normal((B, S, H), dtype=np.float32)
    pr = rng.integers(0, S, size=(B, P, 2)).astype(np.int32)
    w1_ = (rng.standard_normal((2 * H, H), dtype=np.float32) / np.sqrt(2 * H))
    b1_ = rng.standard_normal(H).astype(np.float32) * 0.1
    w2_ = (rng.standard_normal((H, 2), dtype=np.float32) / np.sqrt(H))
    b2_ = rng.standard_normal(2).astype(np.float32) * 0.1
    out = kernel(hidden_states=hs, pairs=pr, W1=w1_.astype(np.float32), b1=b1_,
                 W2=w2_.astype(np.float32), b2=b2_)
    import scipy.special as sp

    e = np.concatenate([hs[np.arange(B)[:, None], pr[:, :, 0]],
                        hs[np.arange(B)[:, None], pr[:, :, 1]]], -1)
    hpre = e @ w1_ + b1_
    hh = 0.5 * hpre * (1 + sp.erf(hpre / np.sqrt(2)))
    exp = hh @ w2_ + b2_
    err = np.linalg.norm(out - exp) / np.linalg.norm(exp)
    print("self-check rel err:", err)



# revision 4
# speedup vs baseline: 1.0368x; 1.0368x over previous
"""Trainium2 Bass kernel for ContextWindowPredictor.

Computation (per batch b):
    e1 = hidden[b][pairs[b,:,0]]          # (P, H) gather
    e2 = hidden[b][pairs[b,:,1]]          # (P, H) gather
    h  = gelu([e1 e2] @ W1 + b1)          # (P, H)
    out = h @ W2 + b2                     # (P, 2)

Sharding: data-parallel over batch, one batch element per NeuronCore.

Device strategy (v7), token-factored U/V with transposed stage 2 and
fp8 hi/lo DoubleRow stage-1 matmuls:
    h[p] = gelu(U[s0_p] + V[s1_p]),  U = hid @ W1[:H] + b1, V = hid @ W1[H:]

  stage 1: hid.T is prepared on the HOST (input marshalling), split into
           e4m3 hi + residual lo; W1 is scaled by 16 and split the same
           way.  Each psum group runs 12 fp8 DoubleRow passes (157 TF/s):
           8 type-1 passes (hid_hi, hid_lo) x (Wh, Wh) and 4 type-2
           passes (hid_hi[2i], hid_hi[2i+1]) x (Wl[2i], Wl[2i+1]) — the
           hi*lo cross term is dropped (~1e-4 relative).  This keeps
           bf16-level accuracy at 0.75x the bf16 cycle count.  U,V live
           in SBUF by h-half ([128, 16, 512]: token s -> partition s%128,
           rank-stripe s//128) holding 16x-scaled values; 16*b1 is folded
           into the U psum->SBUF copies (DVE); V copies on ACT.
  stage 2: SBUF-source TRANSPOSE-mode dma_gather pulls pair rows out of
           U/V into [h-partition, h-tile, pair] layout, 512 pairs per
           call (the SWDGE ring cannot take 1024-descriptor gathers), as
           soon as each half's U (resp. V) is written.  The h0 gathers/
           adds/gelus and the h1 e1-gathers all run UNDER the remaining
           stage-1 matmuls; SBUF pools are ordered so the tail tiles
           reuse addresses that die early.  e1+e2 on DVE in-place, Gelu
           (exact erf) on ACT in-place with scale=1/16 undoing the W1
           scaling, then W2 on the PE as [2, 512] matmuls (contraction
           over h on partitions) into a single 8-bank psum tile, copied
           and DMA'd out once.  b2 is added on the host while unsharding.
"""

import sys

import numpy as np

if "/opt/trn_rl_repo" not in sys.path:
    sys.path.insert(0, "/opt/trn_rl_repo")

B, S, H, P = 8, 2048, 1024, 4096
N_CORES = 8
ST = S // 128          # 16 token tiles
KT = H // 128          # 8 contraction tiles per W1 half
NI = 512               # pairs per W2 psum chunk
NC = P // NI           # 8 chunks
NQ = 8                 # gather waves (512 idx max per SWDGE gather)
QP = P // NQ           # 512 pairs per gather call

_CACHE: dict = {}


def _build(rw=(16,) * NQ, act_name: str = "Gelu"):
    import concourse.bacc as bacc
    import concourse.mybir as mybir
    from concourse.tile import TileContext

    dt = mybir.dt
    AF = mybir.ActivationFunctionType
    PM = mybir.MatmulPerfMode

    nc = bacc.Bacc("TRN2", target_bir_lowering=False)

    # hid8[p, st, kt, d, q]: d=0 -> e4m3(hid), d=1 -> e4m3 residual;
    # hid8[p, st, kt, d, q] ~ hid[st*128+q, kt*128+p] split hi/lo
    hid8 = nc.dram_tensor("hid8", [128, ST, KT, 2, 128], dt.float8e4,
                          kind="ExternalInput")
    # w1t1[p,half,hc,kt,d,j]: both d slots = e4m3(16*W1) block (kt, hc)
    w1t1 = nc.dram_tensor("w1t1", [128, 2, 2, KT, 2, 512], dt.float8e4,
                          kind="ExternalInput")
    # w1t2[p,half,hc,i,d,j]: slot d = e4m3 residual of 16*W1, k-tile 2i+d
    w1t2 = nc.dram_tensor("w1t2", [128, 2, 2, KT // 2, 2, 512], dt.float8e4,
                          kind="ExternalInput")
    b1r = nc.dram_tensor("b1r", [128, H], dt.bfloat16, kind="ExternalInput")
    # w2s[p, kt, o] = W2[kt*128+p, o]
    w2s = nc.dram_tensor("w2s", [128, KT, 2], dt.bfloat16, kind="ExternalInput")
    idx0 = nc.dram_tensor("idx0", [128, P // 16], dt.int16, kind="ExternalInput")
    idx1 = nc.dram_tensor("idx1", [128, P // 16], dt.int16, kind="ExternalInput")
    # outT[p, c, o] = logits[c * 128 + p, o] (pairs in sorted order)
    outT = nc.dram_tensor("outT", [128, P // 128, 2], dt.float32,
                          kind="ExternalOutput")

    act_fn = getattr(AF, act_name)

    with TileContext(nc) as tc:
        with (
            tc.tile_pool(name="uv", bufs=1) as uvp,
            tc.tile_pool(name="cst", bufs=1) as cst,
            tc.tile_pool(name="ge0", bufs=1) as ge0p,
            tc.tile_pool(name="w1l", bufs=1) as w1lp,
        ):
            # ---- constants (loaded later where latency allows) ----
            i0s = cst.tile([128, P // 16], dt.int16, tag="i0s")
            i1s = cst.tile([128, P // 16], dt.int16, tag="i1s")
            b1s = cst.tile([128, H], dt.bfloat16, tag="b1s")
            w2t = cst.tile([128, KT, 2], dt.bfloat16, tag="w2t")

            usb = [uvp.tile([128, ST, 512], dt.bfloat16, tag=f"usb{h}",
                            name=f"usb{h}") for h in range(2)]
            vsb = [uvp.tile([128, ST, 512], dt.bfloat16, tag=f"vsb{h}",
                            name=f"vsb{h}") for h in range(2)]

            # h0 pair tiles; after in-place add + gelu these hold
            # gelu(hpre) h-half-0 until W2 reads them.
            e1h0 = [ge0p.tile([128, 4, QP], dt.bfloat16, tag=f"e1h0q{q}",
                              name=f"e1h0q{q}") for q in range(NQ)]
            # last W1 piece lives to the end of stage 1 — keep it out of the
            # recycled region so tail tiles don't inherit its WAR.
            w1p = {(1, 1): (
                w1lp.tile([128, KT, 2, 512], dt.float8e4, tag="w1pL1",
                          name="w1t1_1_1"),
                w1lp.tile([128, KT // 2, 2, 512], dt.float8e4, tag="w1pL2",
                          name="w1t2_1_1"),
            )}

            def gather(src, isrc, et, q):
                from concourse.bass import AP
                nc.gpsimd.dma_gather(
                    out_ap=et[:],
                    in_ap=src if isinstance(src, AP) else src[:],
                    idxs_ap=isrc[:, q * (QP // 16):(q + 1) * (QP // 16)],
                    num_idxs=QP,
                    num_idxs_reg=QP,
                    elem_size=512,
                    transpose=True,
                    sbuf_tokens_per_rank=128,
                    sbuf_free_dim_per_rank=1024,
                )

            # ================= stage 1 (+ h0 stage-2 front) =================
            # Pool open order controls SBUF placement: e20 (dead ~3/4 in) and
            # s1w (W1 pieces 0-2, dead by the U-h1 section) sit at the bottom
            # of the recycled region; hsb (read until the last matmul) at the
            # top, so the tail's e11/e21/lg pools land on early-dying space.
            with (
                tc.tile_pool(name="e20", bufs=2) as e20p,
                tc.tile_pool(name="s1w", bufs=1) as s1w,
                tc.tile_pool(name="s1h", bufs=1) as s1h,
                tc.tile_pool(name="ps1", bufs=4, space="PSUM") as ps1,
            ):
                # first W1 piece in chunks so matmul 0 starts early;
                # b1 right behind it (needed by the first U copy)
                t0a = s1w.tile([128, KT, 2, 512], dt.float8e4, tag="w1p00a",
                               name="w1t1_0_0")
                nc.scalar.dma_start(out=t0a[:, 0:4, :, :],
                                    in_=w1t1[:, 0, 0, 0:4, :, :])
                hsb = s1h.tile([128, ST, KT, 2, 128], dt.float8e4, tag="hsb")
                nc.sync.dma_start(out=hsb[:, 0, :, :, :], in_=hid8[:, 0, :, :, :])
                nc.scalar.dma_start(out=t0a[:, 4:8, :, :],
                                    in_=w1t1[:, 0, 0, 4:8, :, :])
                t0b = s1w.tile([128, KT // 2, 2, 512], dt.float8e4, tag="w1p00b",
                               name="w1t2_0_0")
                nc.scalar.dma_start(out=t0b[:], in_=w1t2[:, 0, 0, :, :, :])
                nc.scalar.dma_start(out=b1s[:], in_=b1r[:])
                w1p[(0, 0)] = (t0a, t0b)
                for st in range(1, ST):
                    nc.sync.dma_start(out=hsb[:, st, :, :, :],
                                      in_=hid8[:, st, :, :, :])
                for half, hc, tg in ((1, 0, "w1p10"), (0, 1, "w1p01")):
                    ta = s1w.tile([128, KT, 2, 512], dt.float8e4, tag=tg + "a",
                                  name=f"w1t1_{half}_{hc}")
                    nc.scalar.dma_start(out=ta[:], in_=w1t1[:, half, hc, :, :, :])
                    tb = s1w.tile([128, KT // 2, 2, 512], dt.float8e4,
                                  tag=tg + "b", name=f"w1t2_{half}_{hc}")
                    nc.scalar.dma_start(out=tb[:], in_=w1t2[:, half, hc, :, :, :])
                    w1p[(half, hc)] = (ta, tb)
                nc.scalar.dma_start(out=w1p[(1, 1)][0][:],
                                    in_=w1t1[:, 1, 1, :, :, :])
                nc.scalar.dma_start(out=w1p[(1, 1)][1][:],
                                    in_=w1t2[:, 1, 1, :, :, :])
                nc.sync.dma_start(out=i0s[:], in_=idx0[:])
                nc.sync.dma_start(out=i1s[:], in_=idx1[:])
                nc.sync.dma_start(out=w2t[:], in_=w2s[:])

                def s1_section(hc, half):
                    dsts = usb if half == 0 else vsb
                    ta, tb = w1p[(half, hc)]
                    for st in range(ST):
                        ps = ps1.tile([128, 512], dt.float32, tag="ps",
                                      name=f"ps_{hc}_{half}_{st}")
                        # type-1: (hid_hi, hid_lo) x (Wh, Wh), one per k-tile
                        for kt in range(KT):
                            nc.tensor.matmul(
                                ps[:],
                                hsb[:, st, kt, :, :],
                                ta[:, kt, :, :],
                                start=(kt == 0),
                                stop=False,
                                perf_mode=PM.DoubleRow,
                            )
                        # type-2: (hid_hi[2i], hid_hi[2i+1]) x (Wl[2i], Wl[2i+1])
                        for i in range(KT // 2):
                            nc.tensor.matmul(
                                ps[:],
                                hsb[:, st, 2 * i:2 * i + 2, 0, :],
                                tb[:, i, :, :],
                                start=False,
                                stop=(i == KT // 2 - 1),
                                perf_mode=PM.DoubleRow,
                            )
                        dst = dsts[hc][:, st, :]
                        if half == 0:
                            nc.vector.tensor_add(
                                dst, ps[:], b1s[:, hc * 512:(hc + 1) * 512]
                            )
                        else:
                            nc.scalar.activation(dst, ps[:], AF.Copy)

                s1_section(0, 0)
                # e1-h0 gathers fire once usb0 is written (~1/4 into stage 1)
                for q in range(NQ):
                    gather(usb[0], i0s, e1h0[q], q)
                s1_section(0, 1)
                # h0 e2-gathers + adds + gelus run under the h1 matmul sections
                for q in range(NQ):
                    e2t = e20p.tile([128, 4, QP], dt.bfloat16, tag="e2h0",
                                    name=f"e2h0q{q}")
                    gather(vsb[0], i1s, e2t, q)
                    nc.vector.tensor_add(e1h0[q][:], e1h0[q][:], e2t[:])
                    nc.scalar.activation(e1h0[q][:], e1h0[q][:], act_fn, scale=1.0 / 16.0)
                s1_section(1, 0)
                s1_section(1, 1)

            # ================= stage 2 tail: h1 + W2 =================
            with (
                tc.tile_pool(name="e11", bufs=1) as e11p,
                tc.tile_pool(name="e21", bufs=6) as e21p,
                tc.tile_pool(name="ps2", bufs=1, space="PSUM") as ps2p,
                tc.tile_pool(name="lg", bufs=1) as lgp,
            ):
                e1h1 = [e11p.tile([128, 4, QP], dt.bfloat16, tag=f"e1h1q{q}",
                                  name=f"e1h1q{q}") for q in range(NQ)]
                # e1-h1 gathers fire at U-h1 completion, under the V-h1 mms
                for q in range(NQ):
                    gather(usb[1], i0s, e1h1[q], q)
                # issue all e2-h1 gathers first so the Pool queue streams
                # them back-to-back once vsb1 lands
                e2h1 = []
                for q in range(NQ):
                    e2t = e21p.tile([128, 4, QP], dt.bfloat16, tag="e2h1",
                                    name=f"e2h1q{q}")
                    gather(vsb[1][:, 0:rw[q], :], i1s, e2t, q)
                    e2h1.append(e2t)
                # adds first, then gelus, then W2+copy: keeps the lgt psum
                # copies (which wait on W2) from head-of-line-blocking the
                # next wave's add on the in-order DVE queue.
                for q in range(NQ):
                    nc.vector.tensor_add(e1h1[q][:], e1h1[q][:], e2h1[q][:])
                for q in range(NQ):
                    nc.scalar.activation(e1h1[q][:], e1h1[q][:], act_fn, scale=1.0 / 16.0)
                # W2 with PAIRS on the output partitions and o=2 on the free
                # dim: each 128-pair block is 8 accumulating matmuls with
                # out free size 2, so the whole W2 stage is 256 tiny (~4 ns)
                # matmuls instead of 64 free-512 ones (13.6 us -> ~1 us).
                # The pair block is the stationary operand; W2 streams as a
                # 2-row moving operand.
                ps2 = ps2p.tile([128, P // 128, 2], dt.float32, tag="ps2")

                lgt = lgp.tile([128, P // 128, 2], dt.float32, tag="lgt")
                for q in range(NQ):
                    for j in range(QP // 128):
                        b = q * (QP // 128) + j
                        sl = slice(j * 128, (j + 1) * 128)
                        for kt in range(KT):
                            hat = e1h0[q] if kt < 4 else e1h1[q]
                            nc.tensor.matmul(
                                ps2[:, b, :],
                                hat[:, kt % 4, sl],
                                w2t[:, kt, :],
                                start=(kt == 0),
                                stop=(kt == KT - 1),
                            )
                nc.vector.tensor_copy(lgt[:], ps2[:])
                nc.sync.dma_start(out=outT[:], in_=lgt[:])

    nc.compile()
    return nc


def _get_nc(rw=(16,) * NQ):
    key = tuple(rw)
    if key not in _CACHE:
        _CACHE[key] = _build(key)
    return _CACHE[key]


def _wrap_idx(idx: np.ndarray) -> np.ndarray:
    """[P] index list -> [128, P//16] int16 layout dma_gather expects."""
    w = idx.astype(np.int16).reshape(P // 16, 16).T  # [16, P//16]
    return np.ascontiguousarray(np.tile(w, (8, 1)))  # [128, P//16]


def _make_in_maps(hidden_states, pairs, W1, b1, W2):
    import ml_dtypes

    bf16 = ml_dtypes.bfloat16
    e4 = ml_dtypes.float8_e4m3  # IEEE e4m3 (max 240) == TRN FP8_EXP4
    hs = np.asarray(hidden_states, dtype=np.float32)
    pairs_i = np.asarray(pairs).astype(np.int32)
    W1f = np.asarray(W1, dtype=np.float32) * 16.0
    wh = W1f.astype(e4)
    wl = (W1f - wh.astype(np.float32)).astype(e4)
    # [p, half, hc, kt, j] view of a [2H, H] matrix
    def pview(w):
        return w.reshape(2, KT, 128, 2, 512).transpose(2, 0, 3, 1, 4)
    whr = pview(wh)
    w1t1 = np.ascontiguousarray(
        np.stack([whr, whr], axis=4)  # both DoubleRow slots = Wh
    )
    wlr = pview(wl)  # [p, half, hc, kt, j], kt = 2i + d
    w1t2 = np.ascontiguousarray(
        wlr.reshape(128, 2, 2, KT // 2, 2, 512)
    )
    b1f = np.ascontiguousarray(
        np.broadcast_to((np.asarray(b1, dtype=np.float32) * 16.0)
                        .reshape(1, H), (128, H)).astype(bf16)
    )
    w2sv = np.ascontiguousarray(
        np.asarray(W2, dtype=np.float32).reshape(KT, 128, 2).transpose(1, 0, 2)
        .astype(bf16)
    )
    in_maps = []
    for c in range(N_CORES):
        hc32 = hs[c]
        hh = hc32.astype(e4)
        hl = (hc32 - hh.astype(np.float32)).astype(e4)
        hd = np.stack([hh, hl])  # [2, S, H]
        h8 = np.ascontiguousarray(
            hd.reshape(2, ST, 128, KT, 128).transpose(4, 1, 3, 0, 2)
        )
        in_maps.append(
            {
                "hid8": h8,
                "w1t1": w1t1,
                "w1t2": w1t2,
                "b1r": b1f,
                "w2s": w2sv,
                "idx0": _wrap_idx(pairs_i[c, :, 0]),
                "idx1": _wrap_idx(pairs_i[c, :, 1]),
            }
        )
    return in_maps


def kernel(hidden_states, pairs, W1, b1, W2, b2):
    from concourse.bass_utils import run_bass_kernel_spmd

    pairs_i = np.asarray(pairs).astype(np.int32)
    # sort each core's pairs by the e2 token so tail gather wave q only
    # touches the first rw[q] token stripes of V-h1 (progressive deps)
    perms = [np.argsort(pairs_i[c, :, 1], kind="stable")
             for c in range(N_CORES)]
    ps = np.stack([pairs_i[c][perms[c]] for c in range(N_CORES)])
    rw = tuple(
        int(min(16, max(1, int(ps[:, (q + 1) * QP - 1, 1].max()) // 128 + 1)))
        for q in range(NQ)
    )
    nc = _get_nc(rw)
    in_maps = _make_in_maps(hidden_states, ps, W1, b1, W2)
    res = run_bass_kernel_spmd(nc, in_maps, core_ids=list(range(N_CORES)))
    b2f = np.asarray(b2, dtype=np.float32).reshape(1, 2)
    out = np.empty((N_CORES, P, 2), np.float32)
    for c in range(N_CORES):
        sorted_out = (np.asarray(res.results[c]["outT"])
                      .transpose(1, 0, 2).reshape(P, 2))
        out[c, perms[c]] = sorted_out + b2f
    return np.ascontiguousarray(out)


if __name__ == "__main__":
    rng = np.random.default_rng(0)
    hs = rng.standard_normal((B, S, H), dtype=np.float32)
    pr = rng.integers(0, S, size=(B, P, 2)).astype(np.int32)
    w1_ = (rng.standard_normal((2 * H, H), dtype=np.float32) / np.sqrt(2 * H))
    b1_ = rng.standard_normal(H).astype(np.float32) * 0.1
    w2_ = (rng.standard_normal((H, 2), dtype=np.float32) / np.sqrt(H))
    b2_ = rng.standard_normal(2).astype(np.float32) * 0.1
    out = kernel(hidden_states=hs, pairs=pr, W1=w1_.astype(np.float32), b1=b1_,
                 W2=w2_.astype(np.float32), b2=b2_)
    import scipy.special as sp

    e = np.concatenate([hs[np.arange(B)[:, None], pr[:, :, 0]],
                        hs[np.arange(B)[:, None], pr[:, :, 1]]], -1)
    hpre = e @ w1_ + b1_
    hh = 0.5 * hpre * (1 + sp.erf(hpre / np.sqrt(2)))
    exp = hh @ w2_ + b2_
    err = np.linalg.norm(out - exp) / np.linalg.norm(exp)
    print("self-check rel err:", err)

